# revision 1
# baseline (speedup 1.0000x reference)
"""Trainium2 Bass kernel for CausalSelfAttention (B=2, T=2048, D=1024, H=16).

Sharding (8 cores): Megatron-style tensor parallel. Core c owns heads
{2c, 2c+1}: column-parallel c_attn (384 of 3072 output features),
full attention for its 2 heads x 2 batches, row-parallel c_proj
(128 of 1024 contraction rows). Host sums the 8 partial outputs and
adds b_proj.

Device algorithm (per core), all matmuls bf16, softmax f32:
  1. qkv^T = Wslice^T @ x^T   -- x^T arrives pre-transposed bf16 from host.
     q^T, k^T stay in SBUF; v^T tiles are PE-transposed (identity matmul)
     into natural [k, d] layout, rows scaled by the key mask, with the
     0/1 key-mask column appended (col 64/129).
  2. Attention in the TRANSPOSED orientation: S^T[k, q] = k^T.T @ q^T
     per 128-row k-tile; exp(S^T - 10) on ACT straight out of PSUM
     (no max-subtraction: logits are O(1), the -10 shift cancels in the
     softmax ratio); causal handled by skipping invalid column ranges +
     an upper-triangular multiplicative mask on diagonal blocks.
  3. PV: out[65, q] = [v_h | mask01].T @ P^T accumulated over k-tiles.
     Row 64 is the softmax denominator (masked keys excluded via the
     zeroed v rows + mask column). rq = qmask / (denom + eps) is
     broadcast across partitions with a ones-matmul and multiplied in.
  4. out = y^T.T @ Wproj_rows -> partial [4096, 1024] f32, host-summed.
"""

import functools

import numpy as np
import ml_dtypes

import concourse.bass as bass
import concourse.mybir as mybir
import concourse.tile as tile
from concourse import bacc
from concourse.bass_utils import run_bass_kernel_spmd
from concourse.masks import make_upper_triangular, make_identity

BF16 = mybir.dt.bfloat16
F32 = mybir.dt.float32
AF = mybir.ActivationFunctionType
OP = mybir.AluOpType

B, T, D, NH = 2, 2048, 1024, 16
DH = 64                  # head dim
HPC = 2                  # heads per core
NCORES = 8
TT = B * T               # 4096 total tokens
P = 128
KC = D // P              # 8 contraction tiles for qkv
M3 = 3 * HPC * DH // P   # 3 feature tiles (q,k,v each 128 wide)
SPAN = 512               # q-span processed per softmax pass
NSP = T // SPAN          # 4 spans per batch
NKT = T // P             # 16 k-tiles per batch
QSCALE = 1.0 / np.sqrt(DH)
ESHIFT = -10.0           # constant exp shift; cancels in softmax ratio


def build(debug_outs=False):
    nc = bacc.Bacc(None)

    xT = nc.dram_tensor("xT", [D, TT], BF16, kind="ExternalInput")
    wqkv = nc.dram_tensor("wqkv", [KC, P, 3 * P], BF16, kind="ExternalInput")
    bqkv = nc.dram_tensor("bqkv", [P, 3], F32, kind="ExternalInput")
    wproj = nc.dram_tensor("wproj", [P, D], BF16, kind="ExternalInput")
    mrowinv = nc.dram_tensor("mrowinv", [1, TT], F32, kind="ExternalInput")
    mcol = nc.dram_tensor("mcol", [B, P, NKT], F32, kind="ExternalInput")
    out = nc.dram_tensor("out", [TT, D], BF16, kind="ExternalOutput")

    with tile.TileContext(nc) as tc:
        with (
            tc.tile_pool(name="singles", bufs=1) as singles,
            tc.tile_pool(name="stage", bufs=2) as stage,
            tc.tile_pool(name="pt", bufs=4) as ptp,
            tc.tile_pool(name="rows", bufs=2) as rows,
            tc.tile_pool(name="outs", bufs=3) as outs,
            tc.tile_pool(name="ps", bufs=2, space="PSUM") as ps,
            tc.tile_pool(name="ps2", bufs=2, space="PSUM") as ps2,
        ):
            # ---- constants / weights (small DMAs first: casts block on them) ----
            wqkv_sb = singles.tile([P, KC, 3 * P], BF16)
            nc.sync.dma_start(out=wqkv_sb, in_=wqkv.rearrange("k p m -> p k m"))
            bqkv_sb = singles.tile([P, 3], F32)
            nc.sync.dma_start(out=bqkv_sb, in_=bqkv[:, :])
            wproj_sb = singles.tile([P, D], BF16)
            nc.sync.dma_start(out=wproj_sb, in_=wproj[:, :])
            mrowinv_sb = singles.tile([1, TT], F32)
            nc.sync.dma_start(out=mrowinv_sb, in_=mrowinv[:, :])
            mcol_sb = singles.tile([P, B, NKT], F32)
            nc.sync.dma_start(out=mcol_sb, in_=mcol.rearrange("b p o -> p b o"))
            xT_sb = singles.tile([P, KC, TT], BF16)
            for n2 in range(TT // 1024):
                for k in range(KC):
                    tsl = slice(n2 * 1024, (n2 + 1) * 1024)
                    nc.sync.dma_start(out=xT_sb[:, k, tsl],
                                      in_=xT[k * P:(k + 1) * P, tsl])

            eshift_sb = singles.tile([P, 1], F32)
            nc.vector.memset(eshift_sb, ESHIFT)
            ut_sb = singles.tile([P, P], BF16)  # keep q >= k
            make_upper_triangular(nc, ut_sb, val=1.0, diag=True)
            ident = singles.tile([P, P], BF16)
            make_identity(nc, ident)

            qT_sb = singles.tile([P, TT], BF16)   # rows: h0 d0..63 | h1 d0..63
            kT_sb = singles.tile([P, TT], BF16)
            yT_sb = singles.tile([P, TT], BF16)
            v_nat = singles.tile([P, NKT * B, 2 * DH + 2], BF16)
            VW = 2 * DH + 2  # 130

            # ---- phase 1: qkv^T = W^T @ x^T ----
            # groups of [128 feat, 1024 t]; n2-outer so attention can start early
            for n2 in range(TT // 1024):
                for m in range(M3):
                    pq = ps.tile([P, 1024], F32, tag="big")
                    for k in range(KC):
                        for h2 in range(2):
                            nc.tensor.matmul(
                                pq[:, h2 * 512:(h2 + 1) * 512],
                                wqkv_sb[:, k, m * P:(m + 1) * P],
                                xT_sb[:, k, n2 * 1024 + h2 * 512: n2 * 1024 + (h2 + 1) * 512],
                                start=(k == 0), stop=(k == KC - 1),
                            )
                    tcols = slice(n2 * 1024, (n2 + 1) * 1024)
                    if m == 0:
                        nc.scalar.activation(
                            qT_sb[:, tcols], pq[:], AF.Identity,
                            bias=bqkv_sb[:, 0:1], scale=QSCALE)
                    elif m == 1:
                        nc.scalar.activation(
                            kT_sb[:, tcols], pq[:], AF.Identity,
                            bias=bqkv_sb[:, 1:2], scale=1.0)
                    else:
                        vst = stage.tile([P, 1024], BF16, tag="vst")
                        nc.scalar.activation(
                            vst[:], pq[:], AF.Identity,
                            bias=bqkv_sb[:, 2:3], scale=1.0)
                        # phase 2: v natural [k, d] via PE transpose + key mask
                        for jj in range(1024 // P):
                            j32 = n2 * 8 + jj
                            b, o = divmod(j32, NKT)
                            vtp = ps2.tile([P, P], BF16, tag="vtp")
                            nc.tensor.transpose(
                                vtp[:], vst[:, jj * P:(jj + 1) * P], ident[:])
                            nc.vector.tensor_scalar_mul(
                                v_nat[:, j32, 0:DH], vtp[:, 0:DH],
                                mcol_sb[:, b, o:o + 1])
                            nc.vector.tensor_scalar_mul(
                                v_nat[:, j32, DH + 1:2 * DH + 1],
                                vtp[:, DH:2 * DH], mcol_sb[:, b, o:o + 1])
                            nc.vector.tensor_copy(out=v_nat[:, j32, DH:DH + 1],
                                                  in_=mcol_sb[:, b, o:o + 1])
                            nc.vector.tensor_copy(out=v_nat[:, j32, VW - 1:VW],
                                                  in_=mcol_sb[:, b, o:o + 1])

            # ---- phase 3: attention, transposed orientation ----
            def emit_proj(tt):
                ob = outs.tile([P, D], BF16, tag="ob")
                for half in range(2):
                    po = ps2.tile([P, 512], F32, tag="vtp", name="po")
                    nc.tensor.matmul(
                        po[:],
                        yT_sb[:, tt * P:(tt + 1) * P],
                        wproj_sb[:, half * 512:(half + 1) * 512],
                        start=True, stop=True,
                    )
                    nc.vector.tensor_copy(out=ob[:, half * 512:(half + 1) * 512],
                                          in_=po[:])
                nc.sync.dma_start(out=out[tt * P:(tt + 1) * P, :], in_=ob)

            for b in range(B):
                for s in range(NSP):
                    qg = b * T + s * SPAN          # global q col base
                    njs = 4 * s + 4                # k-tiles for this span
                    pvs = [ps.tile([DH + 1, SPAN], F32, tag="pv", name=f"pv{_h}")
                           for _h in range(HPC)]
                    for jj in range(0, njs, 2):
                        sts, pts, offs = [], [], []
                        for h in range(HPC):
                            sts.append(ps.tile([P, 1024], F32, tag="big",
                                               name=f"st{h}"))
                            pts.append(ptp.tile([P, 1024], BF16, tag="pt",
                                                name=f"pt{h}"))
                        for dj in range(2):
                            j = jj + dj
                            off = max(0, j - 4 * s) * P
                            offs.append(off)
                            kb = b * T + j * P
                            for h in range(HPC):
                                hb = h * DH
                                nc.tensor.matmul(
                                    sts[h][:, dj * 512 + off:(dj + 1) * 512],
                                    kT_sb[hb:hb + DH, kb:kb + P],
                                    qT_sb[hb:hb + DH, qg + off:qg + SPAN],
                                    start=True, stop=True,
                                )
                        for h in range(HPC):
                            if offs[1] == 0:
                                nc.scalar.activation(
                                    pts[h][:], sts[h][:], AF.Exp, bias=eshift_sb[:])
                            else:
                                for dj in range(2):
                                    csl = slice(dj * 512 + offs[dj], (dj + 1) * 512)
                                    nc.scalar.activation(
                                        pts[h][:, csl], sts[h][:, csl],
                                        AF.Exp, bias=eshift_sb[:])
                        for dj in range(2):
                            j = jj + dj
                            off = offs[dj]
                            for h in range(HPC):
                                if j >= 4 * s:  # diagonal block: keep q >= k
                                    dsl = slice(dj * 512 + off, dj * 512 + off + P)
                                    nc.vector.tensor_tensor(
                                        pts[h][:, dsl], pts[h][:, dsl], ut_sb[:],
                                        OP.mult)
                                vc0 = h * (DH + 1)
                                nc.tensor.matmul(
                                    pvs[h][:, off:SPAN],
                                    v_nat[:, b * NKT + j, vc0:vc0 + DH + 1],
                                    pts[h][:, dj * 512 + off:(dj + 1) * 512],
                                    start=(j == 0), stop=(j == njs - 1),
                                )
                    for h in range(HPC):
                        den = rows.tile([1, SPAN], F32, tag="den")
                        nc.vector.tensor_tensor(
                            den, pvs[h][DH:DH + 1, :],
                            mrowinv_sb[0:1, qg:qg + SPAN], OP.add)
                        rq = rows.tile([1, SPAN], F32, tag="rq")
                        nc.vector.reciprocal_approx_fast(out=rq, in_=den)
                        bc_sb = rows.tile([DH, SPAN], F32, tag="bcs")
                        nc.gpsimd.partition_broadcast(bc_sb[:], rq[:])
                        hb = h * DH
                        nc.vector.tensor_tensor(
                            yT_sb[hb:hb + DH, qg:qg + SPAN],
                            pvs[h][0:DH, :], bc_sb[:], OP.mult)
                    for tt in range(qg // P, (qg + SPAN) // P):
                        emit_proj(tt)


            if debug_outs:
                d_ut = nc.dram_tensor("d_ut", [P, P], BF16, kind="ExternalOutput")
                d_qT = nc.dram_tensor("d_qT", [P, TT], BF16, kind="ExternalOutput")
                d_kT = nc.dram_tensor("d_kT", [P, TT], BF16, kind="ExternalOutput")
                d_yT = nc.dram_tensor("d_yT", [P, TT], BF16, kind="ExternalOutput")
                d_vn = nc.dram_tensor("d_vn", [P, NKT * B * VW], BF16,
                                      kind="ExternalOutput")
                nc.sync.dma_start(out=d_ut[:, :], in_=ut_sb)
                nc.sync.dma_start(out=d_qT[:, :], in_=qT_sb)
                nc.sync.dma_start(out=d_kT[:, :], in_=kT_sb)
                nc.sync.dma_start(out=d_yT[:, :], in_=yT_sb)
                nc.sync.dma_start(
                    out=d_vn.rearrange("p (j w) -> p j w", w=VW), in_=v_nat)


    nc.finalize()
    return nc


@functools.lru_cache(maxsize=1)
def _built():
    return build()


def _prep_core(c, x, attention_mask, W_attn, b_attn, W_proj):
    bf = ml_dtypes.bfloat16
    q0 = c * HPC * DH
    qs = slice(q0, q0 + P)
    ks = slice(D + q0, D + q0 + P)
    vs = slice(2 * D + q0, 2 * D + q0 + P)
    wsl = np.concatenate(
        [W_attn[:, qs], W_attn[:, ks], W_attn[:, vs]], axis=1)  # [1024, 384]
    bq = b_attn[qs] * QSCALE
    return {
        "wqkv": np.ascontiguousarray(wsl.reshape(KC, P, 3 * P)).astype(bf),
        "bqkv": np.ascontiguousarray(
            np.stack([bq, b_attn[ks], b_attn[vs]], axis=1)).astype(np.float32),
        "wproj": np.ascontiguousarray(W_proj[qs, :]).astype(bf),
    }


def build_in_maps(x, attention_mask, W_attn, b_attn, W_proj):
    bf = ml_dtypes.bfloat16
    x = np.asarray(x, dtype=np.float32)
    attention_mask = np.asarray(attention_mask)
    W_attn = np.asarray(W_attn, dtype=np.float32)
    b_attn = np.asarray(b_attn, dtype=np.float32)
    W_proj = np.asarray(W_proj, dtype=np.float32)

    xT = np.ascontiguousarray(x.reshape(TT, D).T).astype(bf)
    maskf = attention_mask.astype(np.float32)
    mrowinv = np.ascontiguousarray(
        ((1.0 - maskf) * 1e30 + 1e-20).reshape(1, TT)).astype(np.float32)
    mcol = np.ascontiguousarray(
        maskf.reshape(B, NKT, P).transpose(0, 2, 1)).astype(np.float32)  # [B, P, NKT]

    in_maps = []
    for c in range(NCORES):
        m = _prep_core(c, x, attention_mask, W_attn, b_attn, W_proj)
        m["xT"] = xT
        m["mrowinv"] = mrowinv
        m["mcol"] = mcol
        in_maps.append(m)
    return in_maps


def kernel(x, attention_mask, W_attn, b_attn, W_proj, b_proj):
    b_proj = np.asarray(b_proj, dtype=np.float32)
    nc = _built()
    in_maps = build_in_maps(x, attention_mask, W_attn, b_attn, W_proj)
    res = run_bass_kernel_spmd(nc, in_maps, core_ids=list(range(NCORES)))
    acc = np.zeros((TT, D), dtype=np.float32)
    for c in range(NCORES):
        acc += res.results[c]["out"].astype(np.float32)
    acc += b_proj[None, :]
    return acc.reshape(B, T, D)



# revision 3
# speedup vs baseline: 1.1785x; 1.1785x over previous
"""Trainium2 Bass kernel for CausalSelfAttention (B=2, T=2048, D=1024, H=16).

Sharding (8 cores): Megatron-style tensor parallel. Core c owns heads
{2c, 2c+1}: column-parallel c_attn (384 of 3072 output features),
full attention for its 2 heads x 2 batches, row-parallel c_proj
(128 of 1024 contraction rows). Host sums the 8 partial outputs and
adds b_proj.

Device algorithm (per core), all matmuls bf16, softmax f32:
  1. qkv^T = Wslice^T @ x^T   -- x^T arrives pre-transposed bf16 from host.
     q^T, k^T stay in SBUF; v^T tiles are PE-transposed (identity matmul)
     into natural [k, d] layout with a constant ones column appended
     (softmax denominator rides the PV matmul as row 64).
  2. Attention in the TRANSPOSED orientation, one 128-key tile at a time:
     S^T[k, q] = k^T.T @ q^T into a single-bank [128,512] f32 PSUM tile;
     the two heads' S matmuls run CONCURRENTLY in the PE array (row
     groups 0-63 / 64-127 via auto tile_position). exp on ACT straight
     out of PSUM with a per-partition bias column that carries both the
     -10 shift (cancels in the softmax ratio; logits are O(1), so no
     max-subtraction) and the additive key-mask (-50 for masked keys,
     exp -> 0). Causal: skip invalid column ranges + an upper-triangular
     multiplicative mask on diagonal blocks (DVE). The 4-slot one-bank
     PSUM rotation (shared with the qkv-phase accumulators) lets S of
     tile j+1 overlap exp of tile j.
  3. PV: out[65, q] = [v_h | ones].T @ P^T accumulated over k-tiles.
     Row 64 is the denominator (masked keys contribute exp(-60)~0).
     rq = qmask / (denom + eps) broadcast across partitions on GpSimd,
     multiplied into y^T on DVE.
  4. out = y^T.T @ Wproj_rows -> f32 PSUM, DVE cast to bf16 SBUF, DMA.
     Host sums the 8 partials + b_proj.
"""

import functools

import numpy as np
import ml_dtypes

import concourse.bass as bass
import concourse.mybir as mybir
import concourse.tile as tile
from concourse import bacc
from concourse.bass_utils import run_bass_kernel_spmd
from concourse.masks import make_upper_triangular, make_identity

BF16 = mybir.dt.bfloat16
F32 = mybir.dt.float32
AF = mybir.ActivationFunctionType
OP = mybir.AluOpType

B, T, D, NH = 2, 2048, 1024, 16
DH = 64                  # head dim
HPC = 2                  # heads per core
NCORES = 8
TT = B * T               # 4096 total tokens
P = 128
KC = D // P              # 8 contraction tiles for qkv
SPAN = 512               # q-span processed per softmax pass
NSP = T // SPAN          # 4 spans per batch
NKT = T // P             # 16 k-tiles per batch
QSCALE = 1.0 / np.sqrt(DH)
ESHIFT = -10.0           # constant exp shift; cancels in softmax ratio
MASKP = -50.0            # additive key-mask penalty (pre-exp)
VW = 2 * DH + 2          # v_nat width: [v_h0 | 1 | v_h1 | 1]


def build():
    nc = bacc.Bacc(None)

    xT = nc.dram_tensor("xT", [D, TT], BF16, kind="ExternalInput")
    wqkv = nc.dram_tensor("wqkv", [P, KC, 3 * P], BF16, kind="ExternalInput")
    bqkv = nc.dram_tensor("bqkv", [P, 3], F32, kind="ExternalInput")
    wproj = nc.dram_tensor("wproj", [P, D], BF16, kind="ExternalInput")
    mrowinv = nc.dram_tensor("mrowinv", [1, TT], F32, kind="ExternalInput")
    mbias = nc.dram_tensor("mbias", [P, B, NKT], F32, kind="ExternalInput")
    out = nc.dram_tensor("out", [TT, D], BF16, kind="ExternalOutput")

    with tile.TileContext(nc) as tc:
        with (
            tc.tile_pool(name="singles", bufs=1) as singles,
            tc.tile_pool(name="stage", bufs=3) as stage,
            tc.tile_pool(name="pt", bufs=6) as ptp,
            tc.tile_pool(name="rows", bufs=2) as rows,
            tc.tile_pool(name="outs", bufs=3) as outs,
            # 4 one-bank slots shared by qkv accumulators and S^T tiles
            tc.tile_pool(name="psA", bufs=4, space="PSUM") as psA,
            # 2 one-bank pv accumulators + 2 one-bank slots (transpose/proj)
            tc.tile_pool(name="psB", bufs=2, space="PSUM") as psB,
        ):
            # ---- constants / weights (small DMAs first: casts block on them) ----
            wqkv_sb = singles.tile([P, KC, 3 * P], BF16)
            nc.sync.dma_start(out=wqkv_sb, in_=wqkv[:, :, :])
            bqkv_sb = singles.tile([P, 3], F32)
            nc.sync.dma_start(out=bqkv_sb, in_=bqkv[:, :])
            wproj_sb = singles.tile([P, D], BF16)
            nc.sync.dma_start(out=wproj_sb, in_=wproj[:, :])
            mrowinv_sb = singles.tile([1, TT], F32)
            nc.sync.dma_start(out=mrowinv_sb, in_=mrowinv[:, :])
            mbias_sb = singles.tile([P, B, NKT], F32)
            nc.sync.dma_start(out=mbias_sb, in_=mbias[:, :, :])
            xT_sb = singles.tile([P, KC, TT], BF16)
            for n2 in range(TT // 1024):
                for k in range(KC):
                    tsl = slice(n2 * 1024, (n2 + 1) * 1024)
                    nc.sync.dma_start(out=xT_sb[:, k, tsl],
                                      in_=xT[k * P:(k + 1) * P, tsl])

            ut_sb = singles.tile([P, P], BF16)  # keep q >= k
            make_upper_triangular(nc, ut_sb, val=1.0, diag=True)
            ident = singles.tile([P, P], BF16)
            make_identity(nc, ident)

            qT_sb = singles.tile([P, TT], BF16)   # rows: h0 d0..63 | h1 d0..63
            kT_sb = singles.tile([P, TT], BF16)
            yT_sb = singles.tile([P, TT], BF16)
            v_nat = singles.tile([P, NKT * B, VW], BF16)
            # denominator ones columns (64 and 129), constant across tiles
            nc.vector.memset(v_nat[:, :, DH:DH + 1], 1.0)
            nc.vector.memset(v_nat[:, :, VW - 1:VW], 1.0)

            # ---- phase 1: qkv^T = W^T @ x^T ----
            # [128 feat, 512 t] accumulators; n2-outer so attention starts early
            for n2 in range(TT // 1024):
                for m in range(3):
                    for h2 in range(2):
                        pq = psA.tile([P, 512], F32, tag="b1", name="pq")
                        t0 = n2 * 1024 + h2 * 512
                        for k in range(KC):
                            nc.tensor.matmul(
                                pq[:],
                                wqkv_sb[:, k, m * P:(m + 1) * P],
                                xT_sb[:, k, t0:t0 + 512],
                                start=(k == 0), stop=(k == KC - 1),
                            )
                        tcols = slice(t0, t0 + 512)
                        if m == 0:
                            nc.scalar.activation(
                                qT_sb[:, tcols], pq[:], AF.Identity,
                                bias=bqkv_sb[:, 0:1], scale=QSCALE)
                        elif m == 1:
                            nc.scalar.activation(
                                kT_sb[:, tcols], pq[:], AF.Identity,
                                bias=bqkv_sb[:, 1:2], scale=1.0)
                        else:
                            vst = stage.tile([P, 512], BF16, tag="vst")
                            nc.scalar.activation(
                                vst[:], pq[:], AF.Identity,
                                bias=bqkv_sb[:, 2:3], scale=1.0)
                            # phase 2: v natural [k, d] via PE transpose
                            for jj in range(512 // P):
                                j32 = n2 * 8 + h2 * 4 + jj
                                vtp = psB.tile([P, P], BF16, tag="b2")
                                nc.tensor.transpose(
                                    vtp[:], vst[:, jj * P:(jj + 1) * P], ident[:])
                                nc.vector.tensor_copy(
                                    out=v_nat[:, j32, 0:DH], in_=vtp[:, 0:DH])
                                nc.vector.tensor_copy(
                                    out=v_nat[:, j32, DH + 1:2 * DH + 1],
                                    in_=vtp[:, DH:2 * DH])

            # ---- phase 3: attention, transposed orientation ----
            def emit_proj(tt):
                ob = outs.tile([P, D], BF16, tag="ob")
                for half in range(2):
                    po = psB.tile([P, 512], F32, tag="b2", name="po")
                    nc.tensor.matmul(
                        po[:],
                        yT_sb[:, tt * P:(tt + 1) * P],
                        wproj_sb[:, half * 512:(half + 1) * 512],
                        start=True, stop=True,
                    )
                    nc.vector.tensor_copy(out=ob[:, half * 512:(half + 1) * 512],
                                          in_=po[:])
                nc.sync.dma_start(out=out[tt * P:(tt + 1) * P, :], in_=ob)

            for b in range(B):
                for s in range(NSP):
                    qg = b * T + s * SPAN          # global q col base
                    njs = 4 * s + 4                # k-tiles for this span
                    pvs = [psB.tile([DH + 1, SPAN], F32, tag="pv", name=f"pv{_h}")
                           for _h in range(HPC)]
                    for j in range(njs):
                        off = max(0, j - 4 * s) * P
                        kb = b * T + j * P
                        sts, pts = [], []
                        for h in range(HPC):
                            sts.append(psA.tile([P, 512], F32, tag="b1",
                                                name=f"st{h}"))
                            pts.append(ptp.tile([P, 512], BF16, tag="pt",
                                                name=f"pt{h}"))
                        for h in range(HPC):
                            hb = h * DH
                            nc.tensor.matmul(
                                sts[h][:, off:SPAN],
                                kT_sb[hb:hb + DH, kb:kb + P],
                                qT_sb[hb:hb + DH, qg + off:qg + SPAN],
                                start=True, stop=True,
                            )
                        for h in range(HPC):
                            nc.scalar.activation(
                                pts[h][:, off:SPAN], sts[h][:, off:SPAN],
                                AF.Exp, bias=mbias_sb[:, b, j:j + 1])
                        for h in range(HPC):
                            if j >= 4 * s:  # diagonal block: keep q >= k
                                nc.vector.tensor_tensor(
                                    pts[h][:, off:off + P], pts[h][:, off:off + P],
                                    ut_sb[:], OP.mult)
                            vc0 = h * (DH + 1)
                            nc.tensor.matmul(
                                pvs[h][:, off:SPAN],
                                v_nat[:, b * NKT + j, vc0:vc0 + DH + 1],
                                pts[h][:, off:SPAN],
                                start=(j == 0), stop=(j == njs - 1),
                            )
                    for h in range(HPC):
                        den = rows.tile([1, SPAN], F32, tag="den")
                        nc.vector.tensor_tensor(
                            den, pvs[h][DH:DH + 1, :],
                            mrowinv_sb[0:1, qg:qg + SPAN], OP.add)
                        rq = rows.tile([1, SPAN], F32, tag="rq")
                        nc.vector.reciprocal_approx_fast(out=rq, in_=den)
                        bc_sb = rows.tile([DH, SPAN], F32, tag="bcs")
                        nc.gpsimd.partition_broadcast(bc_sb[:], rq[:])
                        hb = h * DH
                        nc.vector.tensor_tensor(
                            yT_sb[hb:hb + DH, qg:qg + SPAN],
                            pvs[h][0:DH, :], bc_sb[:], OP.mult)
                    for tt in range(qg // P, (qg + SPAN) // P):
                        emit_proj(tt)

    nc.finalize()
    return nc


@functools.lru_cache(maxsize=1)
def _built():
    return build()


def _prep_core(c, x, attention_mask, W_attn, b_attn, W_proj):
    bf = ml_dtypes.bfloat16
    q0 = c * HPC * DH
    qs = slice(q0, q0 + P)
    ks = slice(D + q0, D + q0 + P)
    vs = slice(2 * D + q0, 2 * D + q0 + P)
    wsl = np.concatenate(
        [W_attn[:, qs], W_attn[:, ks], W_attn[:, vs]], axis=1)  # [1024, 384]
    bq = b_attn[qs] * QSCALE
    # [P, KC, 3P]: partition-major so the DMA is contiguous per partition
    wq = wsl.reshape(KC, P, 3 * P).transpose(1, 0, 2)
    return {
        "wqkv": np.ascontiguousarray(wq).astype(bf),
        "bqkv": np.ascontiguousarray(
            np.stack([bq, b_attn[ks], b_attn[vs]], axis=1)).astype(np.float32),
        "wproj": np.ascontiguousarray(W_proj[qs, :]).astype(bf),
    }


def build_in_maps(x, attention_mask, W_attn, b_attn, W_proj):
    bf = ml_dtypes.bfloat16
    x = np.asarray(x, dtype=np.float32)
    attention_mask = np.asarray(attention_mask)
    W_attn = np.asarray(W_attn, dtype=np.float32)
    b_attn = np.asarray(b_attn, dtype=np.float32)
    W_proj = np.asarray(W_proj, dtype=np.float32)

    xT = np.ascontiguousarray(x.reshape(TT, D).T).astype(bf)
    maskf = attention_mask.astype(np.float32)
    mrowinv = np.ascontiguousarray(
        ((1.0 - maskf) * 1e30 + 1e-20).reshape(1, TT)).astype(np.float32)
    # per-key exp bias: ESHIFT, plus MASKP for masked keys (exp -> ~0)
    mb = ESHIFT + MASKP * (1.0 - maskf)
    mbias = np.ascontiguousarray(
        mb.reshape(B, NKT, P).transpose(2, 0, 1)).astype(np.float32)  # [P,B,NKT]

    in_maps = []
    for c in range(NCORES):
        m = _prep_core(c, x, attention_mask, W_attn, b_attn, W_proj)
        m["xT"] = xT
        m["mrowinv"] = mrowinv
        m["mbias"] = mbias
        in_maps.append(m)
    return in_maps


def kernel(x, attention_mask, W_attn, b_attn, W_proj, b_proj):
    b_proj = np.asarray(b_proj, dtype=np.float32)
    nc = _built()
    in_maps = build_in_maps(x, attention_mask, W_attn, b_attn, W_proj)
    res = run_bass_kernel_spmd(nc, in_maps, core_ids=list(range(NCORES)))
    acc = np.zeros((TT, D), dtype=np.float32)
    for c in range(NCORES):
        acc += res.results[c]["out"].astype(np.float32)
    acc += b_proj[None, :]
    return acc.reshape(B, T, D)


# revision 7
# speedup vs baseline: 1.3238x; 1.1232x over previous
"""Trainium2 Bass kernel for CausalSelfAttention (B=2, T=2048, D=1024, H=16).

Sharding (8 cores): Megatron-style tensor parallel. Core c owns heads
{2c, 2c+1}: column-parallel c_attn (384 of 3072 output features),
full attention for its 2 heads x 2 batches, row-parallel c_proj
(128 of 1024 contraction rows). Host sums the 8 partial outputs and
adds b_proj.

Device algorithm (per core), all matmuls bf16, softmax f32:
  1. qkv^T = Wslice^T @ x^T   -- x^T arrives pre-transposed bf16 from host.
     q^T, k^T stay in SBUF; v^T tiles are PE-transposed (identity matmul)
     into natural [k, d] layout with a constant ones column appended
     (softmax denominator rides the PV matmul as row 64).
  2. Attention in the TRANSPOSED orientation, one 128-key tile at a time:
     S^T[k, q] = k^T.T @ q^T into a single-bank [128,512] f32 PSUM tile;
     the two heads' S matmuls run CONCURRENTLY in the PE array (row
     groups 0-63 / 64-127 via auto tile_position). exp on ACT straight
     out of PSUM with a per-partition bias column that carries both the
     -10 shift (cancels in the softmax ratio; logits are O(1), so no
     max-subtraction) and the additive key-mask (-50 for masked keys,
     exp -> 0). Causal: skip invalid column ranges + an upper-triangular
     multiplicative mask on diagonal blocks (DVE). The 4-slot one-bank
     PSUM rotation (shared with the qkv-phase accumulators) lets S of
     tile j+1 overlap exp of tile j.
  3. PV: out[65, q] = [v_h | ones].T @ P^T accumulated over k-tiles.
     Row 64 is the denominator (masked keys contribute exp(-60)~0).
     rq = qmask / (denom + eps) broadcast across partitions on GpSimd,
     multiplied into y^T on DVE.
  4. out = y^T.T @ Wproj_rows -> f32 PSUM, DVE cast to bf16 SBUF, DMA.
     Host sums the 8 partials + b_proj.
"""

import functools

import numpy as np
import ml_dtypes

import concourse.bass as bass
import concourse.mybir as mybir
import concourse.tile as tile
from concourse import bacc
from concourse.bass_utils import run_bass_kernel_spmd
from concourse.masks import make_upper_triangular, make_identity

BF16 = mybir.dt.bfloat16
F32 = mybir.dt.float32
AF = mybir.ActivationFunctionType
OP = mybir.AluOpType

B, T, D, NH = 2, 2048, 1024, 16
DH = 64                  # head dim
HPC = 2                  # heads per core
NCORES = 8
TT = B * T               # 4096 total tokens
P = 128
KC = D // P              # 8 contraction tiles for qkv
SPAN = 512               # q-span processed per softmax pass
NSP = T // SPAN          # 4 spans per batch
NKT = T // P             # 16 k-tiles per batch
QSCALE = 1.0 / np.sqrt(DH)
ESHIFT = -10.0           # constant exp shift; cancels in softmax ratio
MASKP = -50.0            # additive key-mask penalty (pre-exp)
VW = 2 * DH + 2          # v_nat width: [v_h0 | 1 | v_h1 | 1]


def build():
    nc = bacc.Bacc(None)

    xT = nc.dram_tensor("xT", [D, TT], BF16, kind="ExternalInput")
    wqkv = nc.dram_tensor("wqkv", [P, KC, 3 * P], BF16, kind="ExternalInput")
    bqkv = nc.dram_tensor("bqkv", [P, 3], F32, kind="ExternalInput")
    wproj = nc.dram_tensor("wproj", [P, D], BF16, kind="ExternalInput")
    mrowinv = nc.dram_tensor("mrowinv", [1, TT], F32, kind="ExternalInput")
    mbias = nc.dram_tensor("mbias", [P, B, NKT], F32, kind="ExternalInput")
    out = nc.dram_tensor("out", [TT, D], BF16, kind="ExternalOutput")

    with tile.TileContext(nc) as tc:
        with (
            tc.tile_pool(name="singles", bufs=1) as singles,
            tc.tile_pool(name="stage", bufs=3) as stage,
            tc.tile_pool(name="pt", bufs=6) as ptp,
            tc.tile_pool(name="rows", bufs=2) as rows,
            tc.tile_pool(name="outs", bufs=3) as outs,
            # 2 two-bank slots shared by qkv accumulators and paired S^T tiles
            tc.tile_pool(name="psA", bufs=2, space="PSUM") as psA,
            # 4 one-bank slots shared by pv accumulators, proj out, transposes
            tc.tile_pool(name="psB", bufs=4, space="PSUM") as psB,
        ):
            # ---- constants / weights (small DMAs first: casts block on them) ----
            wqkv_sb = singles.tile([P, KC, 3 * P], BF16)
            nc.sync.dma_start(out=wqkv_sb, in_=wqkv[:, :, :])
            bqkv_sb = singles.tile([P, 3], F32)
            nc.sync.dma_start(out=bqkv_sb, in_=bqkv[:, :])
            wproj_sb = singles.tile([P, D], BF16)
            nc.sync.dma_start(out=wproj_sb, in_=wproj[:, :])
            mrowinv_sb = singles.tile([1, TT], F32)
            nc.sync.dma_start(out=mrowinv_sb, in_=mrowinv[:, :])
            mbias_sb = singles.tile([P, B, NKT], F32)
            nc.sync.dma_start(out=mbias_sb, in_=mbias[:, :, :])
            xT_sb = singles.tile([P, KC, TT], BF16)
            for n2 in range(TT // 1024):
                for k in range(KC):
                    tsl = slice(n2 * 1024, (n2 + 1) * 1024)
                    nc.sync.dma_start(out=xT_sb[:, k, tsl],
                                      in_=xT[k * P:(k + 1) * P, tsl])

            ut_sb = singles.tile([P, P], BF16)  # keep q >= k
            make_upper_triangular(nc, ut_sb, val=1.0, diag=True)
            ident = singles.tile([P, P], BF16)
            make_identity(nc, ident)

            qT_sb = singles.tile([P, TT], BF16)   # rows: h0 d0..63 | h1 d0..63
            kT_sb = singles.tile([P, TT], BF16)
            yT_sb = singles.tile([P, TT], BF16)
            v_nat = singles.tile([P, NKT * B, VW], BF16)
            # denominator ones columns (64 and 129), constant across tiles
            nc.vector.memset(v_nat[:, :, DH:DH + 1], 1.0)
            nc.vector.memset(v_nat[:, :, VW - 1:VW], 1.0)

            # ---- phase 1: qkv^T = W^T @ x^T ----
            # [128 feat, 512 t] accumulators; n2-outer so attention starts early
            for n2 in range(TT // 1024):
                for m in range(3):
                    for h2 in range(2):
                        pq = psA.tile([P, 512], F32, tag="b1", name="pq",
                                      padded_shape=[P, 1024])
                        t0 = n2 * 1024 + h2 * 512
                        for k in range(KC):
                            nc.tensor.matmul(
                                pq[:],
                                wqkv_sb[:, k, m * P:(m + 1) * P],
                                xT_sb[:, k, t0:t0 + 512],
                                start=(k == 0), stop=(k == KC - 1),
                            )
                        tcols = slice(t0, t0 + 512)
                        if m == 0:
                            nc.scalar.activation(
                                qT_sb[:, tcols], pq[:], AF.Identity,
                                bias=bqkv_sb[:, 0:1], scale=QSCALE)
                        elif m == 1:
                            nc.scalar.activation(
                                kT_sb[:, tcols], pq[:], AF.Identity,
                                bias=bqkv_sb[:, 1:2], scale=1.0)
                        else:
                            vst = stage.tile([P, 512], BF16, tag="vst")
                            nc.scalar.activation(
                                vst[:], pq[:], AF.Identity,
                                bias=bqkv_sb[:, 2:3], scale=1.0)
                            # phase 2: v natural [k, d] via PE transpose
                            for jj in range(512 // P):
                                j32 = n2 * 8 + h2 * 4 + jj
                                vtp = psB.tile([P, P], BF16, tag="pv")
                                nc.tensor.transpose(
                                    vtp[:], vst[:, jj * P:(jj + 1) * P], ident[:])
                                nc.vector.tensor_copy(
                                    out=v_nat[:, j32, 0:DH], in_=vtp[:, 0:DH])
                                nc.vector.tensor_copy(
                                    out=v_nat[:, j32, DH + 1:2 * DH + 1],
                                    in_=vtp[:, DH:2 * DH])

            # ---- phase 3: attention, transposed orientation ----
            def emit_proj(tt):
                ob = outs.tile([P, D], BF16, tag="ob")
                for half in range(2):
                    po = psB.tile([P, 512], F32, tag="pv", name="po")
                    nc.tensor.matmul(
                        po[:],
                        yT_sb[:, tt * P:(tt + 1) * P],
                        wproj_sb[:, half * 512:(half + 1) * 512],
                        start=True, stop=True,
                    )
                    nc.vector.tensor_copy(out=ob[:, half * 512:(half + 1) * 512],
                                          in_=po[:])
                nc.sync.dma_start(out=out[tt * P:(tt + 1) * P, :], in_=ob)

            # proj of span s is deferred into span s+1's k-tile loop so the
            # pv-slot rotation never blocks the next span's first PV matmul
            pending = []
            for b in range(B):
                for s in range(NSP):
                    qg = b * T + s * SPAN          # global q col base
                    njs = 4 * s + 4                # k-tiles for this span
                    pvs = [psB.tile([DH + 1, SPAN], F32, tag="pv", name=f"pv{_h}")
                           for _h in range(HPC)]
                    for j in range(njs):
                        off = max(0, j - 4 * s) * P
                        kb = b * T + j * P
                        st2 = psA.tile([P, 2, 512], F32, tag="b1", name="st2")
                        pt2 = ptp.tile([P, 2, 512], BF16, tag="pt", name="pt2")
                        for h in range(HPC):
                            hb = h * DH
                            nc.tensor.matmul(
                                st2[:, h, off:SPAN],
                                kT_sb[hb:hb + DH, kb:kb + P],
                                qT_sb[hb:hb + DH, qg + off:qg + SPAN],
                                start=True, stop=True,
                            )
                        nc.scalar.activation(
                            pt2[:, :, off:SPAN], st2[:, :, off:SPAN],
                            AF.Exp, bias=mbias_sb[:, b, j:j + 1])
                        for h in range(HPC):
                            if j >= 4 * s:  # diagonal block: keep q >= k
                                nc.vector.tensor_tensor(
                                    pt2[:, h, off:off + P], pt2[:, h, off:off + P],
                                    ut_sb[:], OP.mult)
                            vc0 = h * (DH + 1)
                            nc.tensor.matmul(
                                pvs[h][:, off:SPAN],
                                v_nat[:, b * NKT + j, vc0:vc0 + DH + 1],
                                pt2[:, h, off:SPAN],
                                start=(j == 0), stop=(j == njs - 1),
                            )
                        if pending:
                            emit_proj(pending.pop(0))
                    for h in range(HPC):
                        den = rows.tile([1, SPAN], F32, tag="den")
                        nc.vector.tensor_tensor(
                            den, pvs[h][DH:DH + 1, :],
                            mrowinv_sb[0:1, qg:qg + SPAN], OP.add)
                        rq = rows.tile([1, SPAN], F32, tag="rq")
                        nc.vector.reciprocal_approx_fast(out=rq, in_=den)
                        bc_sb = rows.tile([DH, SPAN], F32, tag="bcs")
                        nc.gpsimd.partition_broadcast(bc_sb[:], rq[:])
                        hb = h * DH
                        nc.vector.tensor_tensor(
                            yT_sb[hb:hb + DH, qg:qg + SPAN],
                            pvs[h][0:DH, :], bc_sb[:], OP.mult)
                    pending.extend(range(qg // P, (qg + SPAN) // P))
            for tt in pending:
                emit_proj(tt)

    nc.finalize()
    return nc


@functools.lru_cache(maxsize=1)
def _built():
    return build()


def _prep_core(c, x, attention_mask, W_attn, b_attn, W_proj):
    bf = ml_dtypes.bfloat16
    q0 = c * HPC * DH
    qs = slice(q0, q0 + P)
    ks = slice(D + q0, D + q0 + P)
    vs = slice(2 * D + q0, 2 * D + q0 + P)
    wsl = np.concatenate(
        [W_attn[:, qs], W_attn[:, ks], W_attn[:, vs]], axis=1)  # [1024, 384]
    bq = b_attn[qs] * QSCALE
    # [P, KC, 3P]: partition-major so the DMA is contiguous per partition
    wq = wsl.reshape(KC, P, 3 * P).transpose(1, 0, 2)
    return {
        "wqkv": np.ascontiguousarray(wq).astype(bf),
        "bqkv": np.ascontiguousarray(
            np.stack([bq, b_attn[ks], b_attn[vs]], axis=1)).astype(np.float32),
        "wproj": np.ascontiguousarray(W_proj[qs, :]).astype(bf),
    }


def build_in_maps(x, attention_mask, W_attn, b_attn, W_proj):
    bf = ml_dtypes.bfloat16
    x = np.asarray(x, dtype=np.float32)
    attention_mask = np.asarray(attention_mask)
    W_attn = np.asarray(W_attn, dtype=np.float32)
    b_attn = np.asarray(b_attn, dtype=np.float32)
    W_proj = np.asarray(W_proj, dtype=np.float32)

    xT = np.ascontiguousarray(x.reshape(TT, D).T).astype(bf)
    maskf = attention_mask.astype(np.float32)
    mrowinv = np.ascontiguousarray(
        ((1.0 - maskf) * 1e30 + 1e-20).reshape(1, TT)).astype(np.float32)
    # per-key exp bias: ESHIFT, plus MASKP for masked keys (exp -> ~0)
    mb = ESHIFT + MASKP * (1.0 - maskf)
    mbias = np.ascontiguousarray(
        mb.reshape(B, NKT, P).transpose(2, 0, 1)).astype(np.float32)  # [P,B,NKT]

    in_maps = []
    for c in range(NCORES):
        m = _prep_core(c, x, attention_mask, W_attn, b_attn, W_proj)
        m["xT"] = xT
        m["mrowinv"] = mrowinv
        m["mbias"] = mbias
        in_maps.append(m)
    return in_maps


def kernel(x, attention_mask, W_attn, b_attn, W_proj, b_proj):
    b_proj = np.asarray(b_proj, dtype=np.float32)
    nc = _built()
    in_maps = build_in_maps(x, attention_mask, W_attn, b_attn, W_proj)
    res = run_bass_kernel_spmd(nc, in_maps, core_ids=list(range(NCORES)))
    acc = np.zeros((TT, D), dtype=np.float32)
    for c in range(NCORES):
        acc += res.results[c]["out"].astype(np.float32)
    acc += b_proj[None, :]
    return acc.reshape(B, T, D)


# revision 14
# speedup vs baseline: 1.4936x; 1.1283x over previous
"""Trainium2 Bass kernel for CausalSelfAttention (B=2, T=2048, D=1024, H=16).

Sharding (8 cores): Megatron-style tensor parallel. Core c owns heads
{2c, 2c+1}: column-parallel c_attn (384 of 3072 output features),
full attention for its 2 heads x 2 batches, row-parallel c_proj
(128 of 1024 contraction rows). Host sums the 8 partial outputs and
adds b_proj.

Device algorithm (per core), all matmuls bf16, softmax f32:
  1. qkv^T = Wslice^T @ x^T   -- x^T arrives pre-transposed bf16 from host.
     q^T, k^T stay in SBUF; v^T tiles are PE-transposed (identity matmul)
     into natural [k, d] layout with a constant ones column appended
     (softmax denominator rides the PV matmul as row 64).
  2. Attention in the TRANSPOSED orientation, one 128-key tile at a time:
     S^T[k, q] = k^T.T @ q^T into a single-bank [128,512] f32 PSUM tile;
     the two heads' S matmuls run CONCURRENTLY in the PE array (row
     groups 0-63 / 64-127 via auto tile_position). exp on ACT straight
     out of PSUM with a per-partition bias column that carries both the
     -10 shift (cancels in the softmax ratio; logits are O(1), so no
     max-subtraction) and the additive key-mask (-50 for masked keys,
     exp -> 0). Causal: skip invalid column ranges + an upper-triangular
     multiplicative mask on diagonal blocks (DVE). The 4-slot one-bank
     PSUM rotation (shared with the qkv-phase accumulators) lets S of
     tile j+1 overlap exp of tile j.
  3. PV: out[65, q] = [v_h | ones].T @ P^T accumulated over k-tiles.
     Row 64 is the denominator (masked keys contribute exp(-60)~0).
     rq = qmask / (denom + eps) broadcast across partitions on GpSimd,
     multiplied into y^T on DVE.
  4. out = y^T.T @ Wproj_rows -> f32 PSUM, DVE cast to bf16 SBUF, DMA.
     Host sums the 8 partials + b_proj.
"""

import functools

import numpy as np
import ml_dtypes

import concourse.bass as bass
import concourse.mybir as mybir
import concourse.tile as tile
from concourse import bacc
from concourse.bass_utils import run_bass_kernel_spmd
from concourse.masks import make_upper_triangular, make_identity

BF16 = mybir.dt.bfloat16
F32 = mybir.dt.float32
AF = mybir.ActivationFunctionType
OP = mybir.AluOpType

B, T, D, NH = 2, 2048, 1024, 16
DH = 64                  # head dim
HPC = 2                  # heads per core
NCORES = 8
TT = B * T               # 4096 total tokens
P = 128
KC = D // P              # 8 contraction tiles for qkv
SPAN = 512               # q-span processed per softmax pass
NSP = T // SPAN          # 4 spans per batch
NKT = T // P             # 16 k-tiles per batch
QSCALE = 1.0 / np.sqrt(DH)
ESHIFT = -10.0           # constant exp shift; cancels in softmax ratio
MASKP = -50.0            # additive key-mask penalty (pre-exp)
VW = 2 * DH + 2          # v_nat width: [v_h0 | 1 | v_h1 | 1]
FP8_QKV = True           # qkv projection in fp8e4m3 DoubleRow (2 k-tiles/mm)
K2 = KC // 2             # DoubleRow k-tile pairs
FP8 = mybir.dt.float8e4


def build():
    nc = bacc.Bacc(None)

    if FP8_QKV:
        xT = nc.dram_tensor("xT", [P, K2, 2, TT], FP8, kind="ExternalInput")
        wqkv = nc.dram_tensor("wqkv", [P, K2, 2, 3 * P], FP8,
                              kind="ExternalInput")
    else:
        xT = nc.dram_tensor("xT", [D, TT], BF16, kind="ExternalInput")
        wqkv = nc.dram_tensor("wqkv", [P, KC, 3 * P], BF16, kind="ExternalInput")
    bqkv = nc.dram_tensor("bqkv", [P, 3], F32, kind="ExternalInput")
    wproj = nc.dram_tensor("wproj", [P, D], BF16, kind="ExternalInput")
    mrowinv = nc.dram_tensor("mrowinv", [1, TT], F32, kind="ExternalInput")
    mbias = nc.dram_tensor("mbias", [P, B, NKT], F32, kind="ExternalInput")
    out = nc.dram_tensor("out", [TT, D], BF16, kind="ExternalOutput")

    with tile.TileContext(nc) as tc:
        with (
            tc.tile_pool(name="singles", bufs=1) as singles,
            tc.tile_pool(name="stage", bufs=3) as stage,
            tc.tile_pool(name="pt", bufs=6) as ptp,
            tc.tile_pool(name="rows", bufs=2) as rows,
            tc.tile_pool(name="outs", bufs=3) as outs,
            # 2 two-bank slots shared by qkv accumulators and paired S^T tiles
            tc.tile_pool(name="psA", bufs=2, space="PSUM") as psA,
            # 4 one-bank slots shared by pv accumulators, proj out, transposes
            tc.tile_pool(name="psB", bufs=4, space="PSUM") as psB,
        ):
            # ---- constants / weights (small DMAs first: casts block on them) ----
            if FP8_QKV:
                wqkv_sb = singles.tile([P, K2, 2, 3 * P], FP8)
                nc.sync.dma_start(out=wqkv_sb, in_=wqkv[:, :, :, :])
            else:
                wqkv_sb = singles.tile([P, KC, 3 * P], BF16)
                nc.sync.dma_start(out=wqkv_sb, in_=wqkv[:, :, :])
            bqkv_sb = singles.tile([P, 3], F32)
            nc.sync.dma_start(out=bqkv_sb, in_=bqkv[:, :])
            wproj_sb = singles.tile([P, D], BF16)
            nc.sync.dma_start(out=wproj_sb, in_=wproj[:, :])
            mrowinv_sb = singles.tile([1, TT], F32)
            nc.sync.dma_start(out=mrowinv_sb, in_=mrowinv[:, :])
            mbias_sb = singles.tile([P, B, NKT], F32)
            nc.sync.dma_start(out=mbias_sb, in_=mbias[:, :, :])
            if FP8_QKV:
                xT_sb = singles.tile([P, K2, 2, TT], FP8)
                for n2 in range(TT // 1024):
                    for k2 in range(K2):
                        tsl = slice(n2 * 1024, (n2 + 1) * 1024)
                        nc.sync.dma_start(out=xT_sb[:, k2, :, tsl],
                                          in_=xT[:, k2, :, tsl])
            else:
                xT_sb = singles.tile([P, KC, TT], BF16)
                for n2 in range(TT // 1024):
                    for k in range(KC):
                        tsl = slice(n2 * 1024, (n2 + 1) * 1024)
                        nc.sync.dma_start(out=xT_sb[:, k, tsl],
                                          in_=xT[k * P:(k + 1) * P, tsl])

            ut_sb = singles.tile([P, P], BF16)  # keep q >= k
            make_upper_triangular(nc, ut_sb, val=1.0, diag=True)
            ident = singles.tile([P, P], BF16)
            make_identity(nc, ident)

            qT_sb = singles.tile([P, TT], BF16)   # rows: h0 d0..63 | h1 d0..63
            kT_sb = singles.tile([P, TT], BF16)
            yT_sb = singles.tile([P, TT], BF16)
            v_nat = singles.tile([P, NKT * B, VW], BF16)
            # denominator ones columns (64 and 129), constant across tiles
            nc.vector.memset(v_nat[:, :, DH:DH + 1], 1.0)
            nc.vector.memset(v_nat[:, :, VW - 1:VW], 1.0)

            # ---- phase 1: qkv^T = W^T @ x^T ----
            # [128 feat, 512 t] accumulators; n2-outer so attention starts early
            for n2 in range(TT // 1024):
                for m in range(3):
                    for h2 in range(2):
                        pq = psA.tile([P, 512], F32, tag="b1", name="pq",
                                      padded_shape=[P, 1024])
                        t0 = n2 * 1024 + h2 * 512
                        if FP8_QKV:
                            for k2 in range(K2):
                                nc.tensor.matmul(
                                    pq[:],
                                    wqkv_sb[:, k2, :, m * P:(m + 1) * P],
                                    xT_sb[:, k2, :, t0:t0 + 512],
                                    start=(k2 == 0), stop=(k2 == K2 - 1),
                                    perf_mode=mybir.MatmulPerfMode.DoubleRow,
                                )
                        else:
                            for k in range(KC):
                                nc.tensor.matmul(
                                    pq[:],
                                    wqkv_sb[:, k, m * P:(m + 1) * P],
                                    xT_sb[:, k, t0:t0 + 512],
                                    start=(k == 0), stop=(k == KC - 1),
                                )
                        tcols = slice(t0, t0 + 512)
                        if m == 0:
                            nc.scalar.activation(
                                qT_sb[:, tcols], pq[:], AF.Identity,
                                bias=bqkv_sb[:, 0:1], scale=QSCALE)
                        elif m == 1:
                            nc.scalar.activation(
                                kT_sb[:, tcols], pq[:], AF.Identity,
                                bias=bqkv_sb[:, 1:2], scale=1.0)
                        else:
                            vst = stage.tile([P, 512], BF16, tag="vst")
                            nc.scalar.activation(
                                vst[:], pq[:], AF.Identity,
                                bias=bqkv_sb[:, 2:3], scale=1.0)
                            # phase 2: v natural [k, d] via PE transpose
                            for jj in range(512 // P):
                                j32 = n2 * 8 + h2 * 4 + jj
                                vtp = psB.tile([P, P], BF16, tag="pv")
                                nc.tensor.transpose(
                                    vtp[:], vst[:, jj * P:(jj + 1) * P], ident[:])
                                nc.vector.tensor_copy(
                                    out=v_nat[:, j32, 0:DH], in_=vtp[:, 0:DH])
                                nc.vector.tensor_copy(
                                    out=v_nat[:, j32, DH + 1:2 * DH + 1],
                                    in_=vtp[:, DH:2 * DH])

            # ---- phase 3: attention, transposed orientation ----
            def emit_proj(tt):
                ob = outs.tile([P, D], BF16, tag="ob")
                for half in range(2):
                    po = psB.tile([P, 512], F32, tag="pv", name="po")
                    nc.tensor.matmul(
                        po[:],
                        yT_sb[:, tt * P:(tt + 1) * P],
                        wproj_sb[:, half * 512:(half + 1) * 512],
                        start=True, stop=True,
                    )
                    nc.vector.tensor_copy(out=ob[:, half * 512:(half + 1) * 512],
                                          in_=po[:])
                nc.sync.dma_start(out=out[tt * P:(tt + 1) * P, :], in_=ob)

            # proj of span s is deferred into span s+1's k-tile loop so the
            # pv-slot rotation never blocks the next span's first PV matmul
            pending = []
            for b in range(B):
                for s in range(NSP):
                    qg = b * T + s * SPAN          # global q col base
                    njs = 4 * s + 4                # k-tiles for this span
                    pvs = [psB.tile([DH + 1, SPAN], F32, tag="pv", name=f"pv{_h}")
                           for _h in range(HPC)]
                    for j in range(njs):
                        off = max(0, j - 4 * s) * P
                        kb = b * T + j * P
                        st2 = psA.tile([P, 2, 512], F32, tag="b1", name="st2")
                        pt2 = ptp.tile([P, 2, 512], BF16, tag="pt", name="pt2")
                        for h in range(HPC):
                            hb = h * DH
                            nc.tensor.matmul(
                                st2[:, h, off:SPAN],
                                kT_sb[hb:hb + DH, kb:kb + P],
                                qT_sb[hb:hb + DH, qg + off:qg + SPAN],
                                start=True, stop=True,
                            )
                        nc.scalar.activation(
                            pt2[:, :, off:SPAN], st2[:, :, off:SPAN],
                            AF.Exp, bias=mbias_sb[:, b, j:j + 1])
                        for h in range(HPC):
                            if j >= 4 * s:  # diagonal block: keep q >= k
                                nc.vector.tensor_tensor(
                                    pt2[:, h, off:off + P], pt2[:, h, off:off + P],
                                    ut_sb[:], OP.mult)
                            vc0 = h * (DH + 1)
                            nc.tensor.matmul(
                                pvs[h][:, off:SPAN],
                                v_nat[:, b * NKT + j, vc0:vc0 + DH + 1],
                                pt2[:, h, off:SPAN],
                                start=(j == 0), stop=(j == njs - 1),
                            )
                        if pending:
                            emit_proj(pending.pop(0))
                    for h in range(HPC):
                        den = rows.tile([1, SPAN], F32, tag="den")
                        nc.vector.tensor_tensor(
                            den, pvs[h][DH:DH + 1, :],
                            mrowinv_sb[0:1, qg:qg + SPAN], OP.add)
                        rq = rows.tile([1, SPAN], F32, tag="rq")
                        nc.vector.reciprocal_approx_fast(out=rq, in_=den)
                        bc_sb = rows.tile([DH, SPAN], F32, tag="bcs")
                        nc.gpsimd.partition_broadcast(bc_sb[:], rq[:])
                        hb = h * DH
                        nc.vector.tensor_tensor(
                            yT_sb[hb:hb + DH, qg:qg + SPAN],
                            pvs[h][0:DH, :], bc_sb[:], OP.mult)
                    pending.extend(range(qg // P, (qg + SPAN) // P))
            for tt in pending:
                emit_proj(tt)

    nc.finalize()
    return nc


@functools.lru_cache(maxsize=1)
def _built():
    return build()


def _prep_core(c, x, attention_mask, W_attn, b_attn, W_proj):
    bf = ml_dtypes.bfloat16
    q0 = c * HPC * DH
    qs = slice(q0, q0 + P)
    ks = slice(D + q0, D + q0 + P)
    vs = slice(2 * D + q0, 2 * D + q0 + P)
    wsl = np.concatenate(
        [W_attn[:, qs], W_attn[:, ks], W_attn[:, vs]], axis=1)  # [1024, 384]
    bq = b_attn[qs] * QSCALE
    if FP8_QKV:
        # [P, K2, 2, 3P]: DoubleRow pairs two 128-row k-tiles per matmul
        wq = wsl.reshape(K2, 2, P, 3 * P).transpose(2, 0, 1, 3)
        wq = np.ascontiguousarray(wq).astype(ml_dtypes.float8_e4m3)
    else:
        # [P, KC, 3P]: partition-major so the DMA is contiguous per partition
        wq = wsl.reshape(KC, P, 3 * P).transpose(1, 0, 2)
        wq = np.ascontiguousarray(wq).astype(bf)
    return {
        "wqkv": wq,
        "bqkv": np.ascontiguousarray(
            np.stack([bq, b_attn[ks], b_attn[vs]], axis=1)).astype(np.float32),
        "wproj": np.ascontiguousarray(W_proj[qs, :]).astype(bf),
    }


def build_in_maps(x, attention_mask, W_attn, b_attn, W_proj):
    bf = ml_dtypes.bfloat16
    x = np.asarray(x, dtype=np.float32)
    attention_mask = np.asarray(attention_mask)
    W_attn = np.asarray(W_attn, dtype=np.float32)
    b_attn = np.asarray(b_attn, dtype=np.float32)
    W_proj = np.asarray(W_proj, dtype=np.float32)

    xr = x.reshape(TT, D).T  # [D, TT]
    if FP8_QKV:
        xT = np.ascontiguousarray(
            xr.reshape(K2, 2, P, TT).transpose(2, 0, 1, 3)
        ).astype(ml_dtypes.float8_e4m3)  # [P, K2, 2, TT]
    else:
        xT = np.ascontiguousarray(xr).astype(bf)
    maskf = attention_mask.astype(np.float32)
    mrowinv = np.ascontiguousarray(
        ((1.0 - maskf) * 1e30 + 1e-20).reshape(1, TT)).astype(np.float32)
    # per-key exp bias: ESHIFT, plus MASKP for masked keys (exp -> ~0)
    mb = ESHIFT + MASKP * (1.0 - maskf)
    mbias = np.ascontiguousarray(
        mb.reshape(B, NKT, P).transpose(2, 0, 1)).astype(np.float32)  # [P,B,NKT]

    in_maps = []
    for c in range(NCORES):
        m = _prep_core(c, x, attention_mask, W_attn, b_attn, W_proj)
        m["xT"] = xT
        m["mrowinv"] = mrowinv
        m["mbias"] = mbias
        in_maps.append(m)
    return in_maps


def kernel(x, attention_mask, W_attn, b_attn, W_proj, b_proj):
    b_proj = np.asarray(b_proj, dtype=np.float32)
    nc = _built()
    in_maps = build_in_maps(x, attention_mask, W_attn, b_attn, W_proj)
    res = run_bass_kernel_spmd(nc, in_maps, core_ids=list(range(NCORES)))
    acc = np.zeros((TT, D), dtype=np.float32)
    for c in range(NCORES):
        acc += res.results[c]["out"].astype(np.float32)
    acc += b_proj[None, :]
    return acc.reshape(B, T, D)
